# revision 8
# baseline (speedup 1.0000x reference)
"""GAT (3-layer DGL-style GATConv) on 8 Trainium2 NeuronCores.

Strategy (graph/data parallel, dst-sharded, v2):
  * dst nodes are assigned to (core, block) bins by a greedy 4-D balancer so
    that every (block, src-chunk) group holds <= 512 edges across all cores
    -> exactly 4 gather subchunks per chunk, 16 per block, ~2% slot padding.
  * Per layer, a "node" launch computes z = h @ Wext (Wext = [W|Wal|War] so
    the attention terms el/er fall out of the same matmul), sharded by
    contiguous node slice.  The host concatenates z, adds the bias b to every
    row (valid because segment-softmax weights sum to 1 per dst), and feeds
    the full table to the "edge" launch split in 4 chunks (int16 gather ids).
  * The edge launch dma_gathers z rows by src, computes
    ex = exp(leakyrelu(el_src + er_dst)) on the Act engine (expanded to all
    F columns per head), scales the gathered rows on the DVE, and aggregates
    numerator and denominator per 128-dst block with one-hot mask matmuls on
    the tensor engine (masks are graph-static and precomputed on the host).
  * Head mean / ReLU / final class softmax run fused in the edge epilogue.

kernel(**inputs) takes the FULL unsharded inputs and returns the FULL
[N, n_classes] float32 output.
"""

import math
import os
from dataclasses import dataclass, field

import numpy as np
import ml_dtypes

BF16 = ml_dtypes.bfloat16
P = 128
NCHUNK = 4  # z-table split so gather indices fit int16
H = 8


# --------------------------------------------------------------------------
# host-side plan: balanced dst->(core,block) packing, slots, gather indices,
# one-hot masks
# --------------------------------------------------------------------------

@dataclass
class Plan:
    n_cores: int
    N: int
    ND: int            # dst nodes per core
    NB: int            # blocks per core
    CH: int            # z-chunk rows
    chunk_rows: list   # rows per z chunk
    g16: object = None      # [NB, NCHUNK] gather num_idxs (cross-core max, pad16)
    scc: object = None      # [NB, NCHUNK] subchunks per chunk group
    off: object = None      # [NB, NCHUNK] subchunk offset within block
    SCb: object = None      # [NB] subchunks per block
    SCmax: int = 0
    iw_off: object = None   # [NB, NCHUNK+1] idx col offsets (per 16-col unit)
    IW: int = 0             # total idx cols
    MK: int = 0             # total mask cols (= sum SCb * P)
    mk_off: object = None   # [NB] mask col offset
    MW: int = 0             # total meta cols (= sum SCb * 16)
    mw_off: object = None   # [NB] meta col offset
    idx: list = field(default_factory=list)       # per core [P, IW] i16
    masks: list = field(default_factory=list)     # per core [P, MK] bf16
    slot_dst: list = field(default_factory=list)  # per core [NB, SCmax, P] i32
    slot_src: list = field(default_factory=list)  # per core [NB, SCmax, P] i32
    row2node: list = field(default_factory=list)  # per core [NB*P] i32


def build_plan(src, dst, N, n_cores):
    src = np.asarray(src).astype(np.int64)
    dst = np.asarray(dst).astype(np.int64)
    ND = N // n_cores
    assert ND * n_cores == N
    CH = (N + NCHUNK - 1) // NCHUNK
    chunk_rows = [min(CH, N - c * CH) for c in range(NCHUNK)]
    NB = max((ND + P - 1) // P, int(math.ceil(ND / P * 1.024)))
    nbins = n_cores * NB
    cap_e = 4 * P  # per-(bin, chunk) edge capacity -> 4 subchunks

    chunk_of = (src // CH).astype(np.int64)
    deg4 = np.zeros((N, NCHUNK), np.int32)
    np.add.at(deg4, (dst, chunk_of), 1)
    order = np.argsort(-deg4.sum(1), kind="stable")

    loads = np.zeros((nbins, NCHUNK), np.int32)
    counts = np.zeros(nbins, np.int32)
    bin_of = np.empty(N, np.int32)
    pos_of = np.empty(N, np.int32)
    for d in order:
        dd = deg4[d]
        nl = loads + dd
        feas = (counts < P) & (nl <= cap_e).all(1)
        if not feas.any():
            feas = counts < P
        score = nl.max(1).astype(np.float32) + counts * 1e-4
        score[~feas] = np.inf
        b = int(np.argmin(score))
        bin_of[d] = b
        pos_of[d] = counts[b]
        loads[b] += dd
        counts[b] += 1

    core_of = bin_of // NB
    blk_of = bin_of % NB

    # per-(core, block, chunk) counts -> shared program geometry
    cnt = np.zeros((n_cores, NB, NCHUNK), np.int64)
    key_all = (core_of[dst] * NB + blk_of[dst]) * NCHUNK + chunk_of
    np.add.at(cnt.reshape(-1), key_all, 1)
    gmax = cnt.max(axis=0)                       # [NB, NCHUNK]
    g16 = (gmax + 15) // 16 * 16
    scc = (g16 + P - 1) // P                     # subchunks per group
    off = np.zeros((NB, NCHUNK), np.int64)
    off[:, 1:] = np.cumsum(scc, axis=1)[:, :-1]
    SCb = scc.sum(axis=1)
    SCmax = int(SCb.max())
    iw = g16 // 16
    iw_off = np.zeros((NB, NCHUNK + 1), np.int64)
    iw_off[:, 1:] = np.cumsum(iw, axis=1)
    blk_iw = iw_off[:, -1]
    blk_iw_off = np.concatenate([[0], np.cumsum(blk_iw)])
    IW = int(blk_iw_off[-1])
    mk_off = np.concatenate([[0], np.cumsum(SCb * P)])
    MK = int(mk_off[-1])
    mw_off = np.concatenate([[0], np.cumsum(SCb * 16)])
    MW = int(mw_off[-1])

    plan = Plan(n_cores, N, ND, NB, CH, chunk_rows,
                g16=g16, scc=scc, off=off, SCb=SCb, SCmax=SCmax,
                iw_off=iw_off, IW=IW, MK=MK, mk_off=mk_off,
                MW=MW, mw_off=mw_off)
    plan.blk_iw_off = blk_iw_off

    for k in range(n_cores):
        m = core_of[dst] == k
        dk = dst[m]
        sk = src[m]
        ck = chunk_of[m]
        bk = blk_of[dk]
        pk = pos_of[dk]

        idx_arr = np.zeros((P, IW), np.int16)
        masks = np.zeros((P, MK), BF16)
        slot_dst = np.full((NB, SCmax, P), -1, np.int32)
        slot_src = np.full((NB, SCmax, P), -1, np.int32)
        row2node = np.full(NB * P, -1, np.int32)

        sel = core_of == k
        row2node[blk_of[sel] * P + pos_of[sel]] = np.nonzero(sel)[0].astype(np.int32)

        keys = (bk * NCHUNK + ck).astype(np.int64)
        srt = np.argsort(keys, kind="stable")
        ks = keys[srt]
        dks = dk[srt]
        sks = sk[srt]
        pks = pk[srt]
        starts = np.searchsorted(ks, np.arange(NB * NCHUNK))
        ends = np.searchsorted(ks, np.arange(NB * NCHUNK) + 1)
        for b in range(NB):
            for c in range(NCHUNK):
                g0, g1 = starts[b * NCHUNK + c], ends[b * NCHUNK + c]
                n = g1 - g0
                G = int(g16[b, c])
                if G == 0:
                    continue
                GC = G // 16
                flat = np.zeros(G, np.int16)
                flat[:n] = (sks[g0:g1] - c * CH).astype(np.int16)
                grid = flat.reshape(GC, 16).T
                i0 = blk_iw_off[b] + iw_off[b, c]
                idx_arr[:, i0:i0 + GC] = np.tile(grid, (8, 1))
                s = np.arange(n)
                kk = int(off[b, c]) + s // P
                pp = s % P
                dr = pks[g0:g1]
                masks[pp, mk_off[b] + kk * P + dr] = 1
                slot_dst[b, kk, pp] = dks[g0:g1]
                slot_src[b, kk, pp] = sks[g0:g1]
        plan.idx.append(idx_arr)
        plan.masks.append(masks)
        plan.slot_dst.append(slot_dst)
        plan.slot_src.append(slot_src)
        plan.row2node.append(row2node)
    return plan


# --------------------------------------------------------------------------
# bass program builders
# --------------------------------------------------------------------------

def _bass_mods():
    import concourse.bass as bass
    import concourse.bacc as bacc
    import concourse.tile as tile
    import concourse.mybir as mybir
    return bass, bacc, tile, mybir


def build_node_program(Din, HF, R, NT):
    """z = hT.T @ Wext.  Wext = [W | Wal | War] so el/er come out of the
    same matmul.  z rows are bf16, width R; el/er go to a separate output.
    Node tiles are processed in pairs to halve the DMA instruction count."""
    bass, bacc, tile, mybir = _bass_mods()
    f32, bf16 = mybir.dt.float32, mybir.dt.bfloat16
    KC = (Din + P - 1) // P
    assert NT % 2 == 0

    nc = bacc.Bacc("TRN2", target_bir_lowering=False, debug=False)
    hT = nc.dram_tensor("hT", [Din, NT * P], bf16, kind="ExternalInput").ap()
    W = nc.dram_tensor("W", [Din, HF + 16], bf16, kind="ExternalInput").ap()
    z_out = nc.dram_tensor("z_out", [NT * P, R], bf16, kind="ExternalOutput").ap()
    eo = nc.dram_tensor("eo", [NT * P, 16], bf16, kind="ExternalOutput").ap()

    with tile.TileContext(nc) as tc:
        from contextlib import ExitStack
        with ExitStack() as ctx:
            cpool = ctx.enter_context(tc.tile_pool(name="const", bufs=1))
            lpool = ctx.enter_context(tc.tile_pool(name="lhs", bufs=4))
            zpool = ctx.enter_context(tc.tile_pool(name="z", bufs=3))
            ppool = ctx.enter_context(tc.tile_pool(name="psum", bufs=2, space="PSUM"))

            W_t = []
            for kc in range(KC):
                K = min(P, Din - kc * P)
                wt = cpool.tile([K, HF + 16], bf16, tag=f"w{kc}")
                nc.sync.dma_start(wt[:], W[kc * P : kc * P + K, :])
                W_t.append(wt)

            zv = z_out.rearrange("(t p) r -> t p r", p=P)
            ev = eo.rearrange("(t p) r -> t p r", p=P)
            for tp in range(NT // 2):
                lhs = []
                for kc in range(KC):
                    K = min(P, Din - kc * P)
                    lh = lpool.tile([K, 2 * P], bf16, tag=f"lh{kc}")
                    nc.sync.dma_start(
                        lh[:], hT[kc * P : kc * P + K, tp * 2 * P : (tp + 1) * 2 * P]
                    )
                    lhs.append(lh)
                zrow = zpool.tile([P, 2, R], bf16, tag="zrow")
                et = zpool.tile([P, 2, 16], bf16, tag="et")
                for j in range(2):
                    ps = ppool.tile([P, HF], f32, tag=f"psz{j}")
                    pe = ppool.tile([P, 16], f32, tag="pse")
                    for kc in range(KC):
                        nc.tensor.matmul(
                            ps[:], lhsT=lhs[kc][:, j * P : (j + 1) * P],
                            rhs=W_t[kc][:, 0:HF],
                            start=(kc == 0), stop=(kc == KC - 1),
                        )
                        nc.tensor.matmul(
                            pe[:], lhsT=lhs[kc][:, j * P : (j + 1) * P],
                            rhs=W_t[kc][:, HF : HF + 16],
                            start=(kc == 0), stop=(kc == KC - 1),
                        )
                    if j == 0:
                        nc.scalar.activation(
                            zrow[:, j, 0:HF], ps[:],
                            mybir.ActivationFunctionType.Copy,
                        )
                    else:
                        nc.vector.tensor_copy(out=zrow[:, j, 0:HF], in_=ps[:])
                    nc.vector.tensor_copy(out=et[:, j, :], in_=pe[:])
                    if R > HF:
                        nc.vector.memset(zrow[:, j, HF:R], 0)
                nc.sync.dma_start(zv[tp * 2 : tp * 2 + 2, :, :].transpose([1, 0, 2]),
                                  zrow[:])
                nc.sync.dma_start(ev[tp * 2 : tp * 2 + 2, :, :].transpose([1, 0, 2]),
                                  et[:])
    nc.compile()
    return nc


def build_edge_program(HF, R, plan, final, n_classes=41):
    """Gather z rows by src, segment-softmax aggregate per dst block.

    meta per block b (cols [mw_off[b] : mw_off[b+1]], bf16):
      [0 : SCb*8]       el per slot (host-gathered el_full[src], -1e4 pads)
      [SCb*8 : SCb*16]  er per slot (host-gathered er_full[dst])
    masks per block b (cols [mk_off[b] : mk_off[b+1]], bf16): one-hot
      [slot_partition, subchunk * P + dst_row] built on the host.
    """
    bass, bacc, tile, mybir = _bass_mods()
    f32, bf16, i16 = mybir.dt.float32, mybir.dt.bfloat16, mybir.dt.int16
    F = HF // H
    NB, SCmax = plan.NB, plan.SCmax
    XDVE = int(os.environ.get("GAT_XDVE", "4"))

    nqueues = int(os.environ.get("GAT_QUEUES", "4"))
    nc = bacc.Bacc("TRN2", target_bir_lowering=False, debug=False,
                   num_swdge_queues=nqueues)
    zc = [
        nc.dram_tensor(f"z{c}", [plan.chunk_rows[c], R], bf16,
                       kind="ExternalInput").ap()
        for c in range(NCHUNK)
    ]
    idx = nc.dram_tensor("idx", [P, plan.IW], i16, kind="ExternalInput").ap()
    meta = nc.dram_tensor("meta", [P, plan.MW], bf16, kind="ExternalInput").ap()
    mks = nc.dram_tensor("mks", [P, plan.MK], bf16, kind="ExternalInput").ap()
    OW = n_classes if final else F
    out = nc.dram_tensor("out", [NB * P, OW], f32, kind="ExternalOutput").ap()

    GBUFS = 4
    with tile.TileContext(nc) as tc:
        from contextlib import ExitStack
        with ExitStack() as ctx:
            gpool = ctx.enter_context(tc.tile_pool(name="gath", bufs=GBUFS))
            kpool = ctx.enter_context(tc.tile_pool(name="mask", bufs=3))
            xpool = ctx.enter_context(tc.tile_pool(name="exf", bufs=2))
            spool = ctx.enter_context(tc.tile_pool(name="small", bufs=3))
            opool = ctx.enter_context(tc.tile_pool(name="outs", bufs=3))
            ppool = ctx.enter_context(tc.tile_pool(name="psum", bufs=3, space="PSUM"))

            def prep_escore(b):
                """meta DMA + e/elr/exb for block b (issued one block early so
                the expand never stalls on the Act queue)."""
                scb = int(plan.SCb[b])
                mt = spool.tile([P, scb * 16], bf16, tag="meta")
                nc.sync.dma_start(
                    mt[:], meta[:, int(plan.mw_off[b]):int(plan.mw_off[b + 1])])
                e_t = spool.tile([P, scb * 8], bf16, tag="e")
                nc.vector.tensor_tensor(
                    out=e_t[:], in0=mt[:, 0:scb * 8], in1=mt[:, scb * 8:scb * 16],
                    op=mybir.AluOpType.add,
                )
                elr = spool.tile([P, scb * 8], bf16, tag="elr")
                nc.vector.scalar_tensor_tensor(
                    out=elr[:], in0=e_t[:], scalar=0.2, in1=e_t[:],
                    op0=mybir.AluOpType.mult, op1=mybir.AluOpType.max,
                )
                exb = spool.tile([P, scb * 8], bf16, tag="exb")
                nc.scalar.activation(
                    exb[:], elr[:], mybir.ActivationFunctionType.Exp
                )
                return elr, exb

            nxt = prep_escore(0)
            for b in range(NB):
                scb = int(plan.SCb[b])
                iw0 = int(plan.blk_iw_off[b])
                iw_b = int(plan.blk_iw_off[b + 1] - iw0)
                idx_t = spool.tile([P, iw_b], i16, tag="idx")
                nc.sync.dma_start(idx_t[:], idx[:, iw0:iw0 + iw_b])
                mk = kpool.tile([P, scb * P], bf16, tag="mk")
                nc.sync.dma_start(
                    mk[:], mks[:, int(plan.mk_off[b]):int(plan.mk_off[b + 1])])

                Zg = gpool.tile([P, SCmax, R], bf16, tag="Zg")
                for c in range(NCHUNK):
                    G = int(plan.g16[b, c])
                    o0 = int(plan.off[b, c])
                    scc = int(plan.scc[b, c])
                    if G == 0:
                        continue
                    if G % P:
                        # pad slots of the last partial subchunk never get
                        # gathered; zero the subchunk (before the gather, which
                        # overwrites the real slots) so ex=0 kills them cleanly
                        nc.vector.memset(Zg[:, o0 + scc - 1, :], 0)
                    nc.gpsimd.dma_gather(
                        Zg[:, o0:o0 + scc, :],
                        zc[c][:],
                        idx_t[:, int(plan.iw_off[b, c]):int(plan.iw_off[b, c + 1])],
                        num_idxs=G,
                        num_idxs_reg=G,
                        elem_size=R,
                        elem_step=R,
                        queue_num=c % nqueues,
                    )
                elr, exb = nxt
                if b + 1 < NB:
                    nxt = prep_escore(b + 1)
                # denominator matmuls need only masks + exb -> issue first so
                # the PE works while the DVE scales z rows
                ps_n = ppool.tile([P, HF], f32, tag="psn")
                ps_s = ppool.tile([P, 8], f32, tag="pss")
                for k in range(scb):
                    nc.tensor.matmul(
                        ps_s[:], lhsT=mk[:, k * P:(k + 1) * P],
                        rhs=exb[:, k * 8:(k + 1) * 8],
                        start=(k == 0), stop=(k == scb - 1),
                    )
                # ex expanded to all F columns per head, in groups of XGRP
                # subchunks so scale/matmul of group g overlaps expand of g+1;
                # the first XDVE subchunks expand on the DVE (copy), the rest
                # on Act (exp)
                exF = xpool.tile([P, SCmax, HF], bf16, tag="exF")
                xd = min(XDVE, scb)
                XGRP = 4
                if xd > 0:
                    nc.vector.tensor_copy(
                        out=exF[:, 0:xd, :].rearrange("p k (h f) -> p k h f", f=F),
                        in_=exb[:, 0:xd * 8]
                        .rearrange("p (k h) -> p k h", h=H)
                        .unsqueeze(3)
                        .to_broadcast([P, xd, H, F]),
                    )
                for g0 in range(xd, scb, XGRP):
                    g1 = min(g0 + XGRP, scb)
                    nc.scalar.activation(
                        exF[:, g0:g1, :].rearrange("p k (h f) -> p k h f", f=F),
                        elr[:, g0 * 8:g1 * 8]
                        .rearrange("p (k h) -> p k h", h=H)
                        .unsqueeze(3)
                        .to_broadcast([P, g1 - g0, H, F]),
                        mybir.ActivationFunctionType.Exp,
                    )
                # scale gathered z rows by ex (in place), one group at a time,
                # chasing each group with its numerator matmuls
                for g0 in range(0, scb, XGRP):
                    g1 = min(g0 + XGRP, scb)
                    nc.vector.tensor_tensor(
                        out=Zg[:, g0:g1, 0:HF],
                        in0=Zg[:, g0:g1, 0:HF],
                        in1=exF[:, g0:g1, :],
                        op=mybir.AluOpType.mult,
                    )
                    for k in range(g0, g1):
                        nc.tensor.matmul(
                            ps_n[:], lhsT=mk[:, k * P:(k + 1) * P],
                            rhs=Zg[:, k, 0:HF],
                            start=(k == 0), stop=(k == scb - 1),
                        )
                srm = spool.tile([P, 8], f32, tag="srm")
                nc.vector.tensor_scalar_max(out=srm[:], in0=ps_s[:], scalar1=1e-12)
                srec = spool.tile([P, 8], f32, tag="srec")
                nc.vector.reciprocal(out=srec[:], in_=srm[:])
                outg = opool.tile([P, HF], f32, tag="outg")
                nc.vector.tensor_tensor(
                    out=outg[:].rearrange("p (h f) -> p h f", f=F),
                    in0=ps_n[:].rearrange("p (h f) -> p h f", f=F),
                    in1=srec[:].unsqueeze(2).to_broadcast([P, H, F]),
                    op=mybir.AluOpType.mult,
                )
                if not final:
                    r = opool.tile([P, HF], f32, tag="r")
                    nc.scalar.activation(
                        r[:], outg[:], mybir.ActivationFunctionType.Relu,
                        scale=0.125,
                    )
                    ht = opool.tile([P, F], f32, tag="ht")
                    nc.vector.reduce_sum(
                        ht[:],
                        r[:].rearrange("p (h f) -> p f h", f=F),
                        axis=mybir.AxisListType.X,
                    )
                    nc.sync.dma_start(out[b * P:(b + 1) * P, :], ht[:])
                else:
                    q = opool.tile([P, n_classes], f32, tag="q")
                    nc.vector.reduce_sum(
                        q[:],
                        outg[:].rearrange("p (h f) -> p f h", f=F),
                        axis=mybir.AxisListType.X,
                    )
                    qm = spool.tile([P, 1], f32, tag="qm")
                    nc.vector.reduce_max(qm[:], q[:], axis=mybir.AxisListType.X)
                    negm = spool.tile([P, 1], f32, tag="negm")
                    nc.vector.tensor_scalar_mul(out=negm[:], in0=qm[:],
                                                scalar1=-0.125)
                    qe = opool.tile([P, n_classes], f32, tag="qe")
                    nc.scalar.activation(
                        qe[:], q[:], mybir.ActivationFunctionType.Exp,
                        bias=negm[:], scale=0.125,
                    )
                    qs = spool.tile([P, 1], f32, tag="qs")
                    nc.vector.reduce_sum(qs[:], qe[:], axis=mybir.AxisListType.X)
                    qsr = spool.tile([P, 1], f32, tag="qsr")
                    nc.vector.reciprocal(out=qsr[:], in_=qs[:])
                    outf = opool.tile([P, n_classes], f32, tag="outf")
                    nc.vector.tensor_single_scalar(
                        out=outf[:], in_=qe[:], scalar=qsr[:],
                        op=mybir.AluOpType.mult,
                    )
                    nc.sync.dma_start(out[b * P:(b + 1) * P, :], outf[:])
    nc.compile()
    return nc


# --------------------------------------------------------------------------
# orchestration
# --------------------------------------------------------------------------

_PROG_CACHE = {}
LAST_RUN_NS = []  # per-launch max-core exec ns when GAT_TRACE=1
LAST_RESULTS = []  # full BassKernelResults per launch when GAT_TRACE=1


def _get_prog(key, builder):
    if key not in _PROG_CACHE:
        _PROG_CACHE[key] = builder()
    return _PROG_CACHE[key]


def _run(nc, in_maps, n_cores):
    if os.environ.get("GAT_SIM", "0") == "1":
        return _run_sim(nc, in_maps)
    from concourse.bass_utils import run_bass_kernel_spmd

    trace = os.environ.get("GAT_TRACE", "0") == "1"
    core_ids = list(range(n_cores))
    res = run_bass_kernel_spmd(
        nc, in_maps, core_ids,
        trace=trace, trace_cores=core_ids if trace else None,
    )
    if trace:
        LAST_RUN_NS.append(res.exec_time_ns)
        LAST_RESULTS.append(res)
    return res.results


def _run_sim(nc, in_maps):
    """CoreSim (functional simulator) execution, one core at a time."""
    from concourse.bass_interp import CoreSim

    results = []
    for im in in_maps:
        sim = CoreSim(nc, trace=False, require_finite=False, require_nnan=False)
        for name, arr in im.items():
            sim.tensor(name)[:] = arr
        sim.simulate(check_with_hw=False)
        out = {}
        for alloc in nc.m.functions[0].allocations:
            import concourse.mybir as mybir
            if (
                isinstance(alloc, mybir.MemoryLocationSet)
                and alloc.kind == "ExternalOutput"
            ):
                name = alloc.memorylocations[0].name
                out[name] = np.array(sim.tensor(name))
        results.append(out)
    return results


def gat_forward(x, src, dst, params, N=None, n_cores=8, n_classes=41):
    """params: list of 3 dicts with W [Din, H*F], al/ar [H, F], b [H, F]."""
    N = N if N is not None else x.shape[0]
    plan = build_plan(src, dst, N, n_cores)
    NB, CH, ND = plan.NB, plan.CH, plan.ND
    NT = 2 * ((ND + 2 * P - 1) // (2 * P))  # node tiles (pairs) per core

    layer_dims = []
    for prm in params:
        Din = prm["W"].shape[0]
        F = prm["al"].shape[1]
        HF = H * F
        R = ((HF * 2 + 255) // 256) * 256 // 2
        layer_dims.append((Din, F, HF, R))

    h = np.asarray(x, np.float32)
    out_final = None
    for li, prm in enumerate(params):
        Din, F, HF, R = layer_dims[li]
        final = li == len(params) - 1

        node_nc = _get_prog(
            ("node", Din, HF, R, NT), lambda: build_node_program(Din, HF, R, NT)
        )
        W = prm["W"].astype(np.float32)
        Wal = np.einsum("khf,hf->kh", W.reshape(Din, H, F), prm["al"])
        War = np.einsum("khf,hf->kh", W.reshape(Din, H, F), prm["ar"])
        Wext = np.concatenate([W, Wal, War], axis=1).astype(BF16)
        in_maps = []
        for k in range(n_cores):
            hk = h[k * ND : (k + 1) * ND]
            hT = np.zeros((Din, NT * P), BF16)
            hT[:, :ND] = hk.T.astype(BF16)
            in_maps.append({"hT": hT, "W": Wext})
        res = _run(node_nc, in_maps, n_cores)

        z_full = np.concatenate(
            [res[k]["z_out"][:ND] for k in range(n_cores)], axis=0
        )
        eo_full = np.concatenate(
            [res[k]["eo"][:ND] for k in range(n_cores)], axis=0
        ).astype(np.float32)
        el_full = eo_full[:, 0:8]
        er_full = eo_full[:, 8:16]
        # fold the bias into the z table (attention weights sum to 1 per dst)
        zb = z_full[:, 0:HF].astype(np.float32)
        zb += prm["b"].reshape(1, HF)
        z_full[:, 0:HF] = zb.astype(BF16)

        edge_nc = _get_prog(
            ("edge", HF, R, final),
            lambda: build_edge_program(plan=plan, HF=HF, R=R, final=final,
                                       n_classes=n_classes),
        )
        in_maps = []
        for k in range(n_cores):
            sd = plan.slot_dst[k]  # [NB, SCmax, P] node ids, -1 pads
            ss = plan.slot_src[k]
            v = sd >= 0
            ele = np.full((NB, plan.SCmax, P, 8), -1e4, np.float32)
            ele[v] = el_full[ss[v]]
            ere = np.zeros((NB, plan.SCmax, P, 8), np.float32)
            ere[v] = er_full[sd[v]]
            meta = np.zeros((P, plan.MW), BF16)
            for b in range(NB):
                scb = int(plan.SCb[b])
                m0 = int(plan.mw_off[b])
                meta[:, m0:m0 + scb * 8] = (
                    ele[b, :scb].transpose(1, 0, 2).reshape(P, scb * 8)
                )
                meta[:, m0 + scb * 8:m0 + scb * 16] = (
                    ere[b, :scb].transpose(1, 0, 2).reshape(P, scb * 8)
                )
            im = {
                "idx": plan.idx[k],
                "meta": meta,
                "mks": plan.masks[k],
            }
            for c in range(NCHUNK):
                im[f"z{c}"] = np.ascontiguousarray(
                    z_full[c * CH : c * CH + plan.chunk_rows[c]]
                )
            in_maps.append(im)
        res = _run(edge_nc, in_maps, n_cores)

        OW = n_classes if final else F
        nxt = np.zeros((N, OW), np.float32)
        for k in range(n_cores):
            r2n = plan.row2node[k]
            v = r2n >= 0
            nxt[r2n[v]] = res[k]["out"][v]
        if final:
            out_final = nxt
        else:
            h = nxt
    return out_final


def kernel(**inputs):
    x = np.asarray(inputs["x"], np.float32)
    src = np.asarray(inputs["src"])
    dst = np.asarray(inputs["dst"])
    params = []
    for i in range(3):
        params.append(
            {
                "W": np.asarray(inputs[f"W{i}"], np.float32),
                "al": np.asarray(inputs[f"al{i}"], np.float32),
                "ar": np.asarray(inputs[f"ar{i}"], np.float32),
                "b": np.asarray(inputs[f"b{i}"], np.float32),
            }
        )
    return gat_forward(x, src, dst, params, N=x.shape[0], n_cores=8,
                       n_classes=params[2]["al"].shape[1]).astype(np.float32)
